# revision 16
# baseline (speedup 1.0000x reference)
"""Trainium2 Bass kernel for the DiagonalSSMBlock problem.

Math (per batch, sharded one batch per core over 8 cores):
    a = -exp(log_neg_real) + i*imag ; a_bar = exp(a) = r * e^{i theta}
    b_bar = ((a_bar-1)/a)[:,None] * B
    Bu_t = b_bar @ u_t                         (complex, state dim 64)
    h_t = a_bar * h_{t-1} + Bu_t               (diagonal complex scan over L)
    y_t = Re(C @ h_t) + D*u_t ; out = LN(u + y) * gamma + beta

The kernel is DMA-bound, so all HBM traffic runs in bf16 (the correctness
gate is rel-err < 2e-2; the all-bf16 pipeline sits at ~2.4e-3):
  * u ships twice from host in bf16: transposed [d, l] for the Bu matmul
    (contraction over d_model needs d on partitions) and natural [l, d] for
    the residual (pre-scaled by (1+D) on host).
  * Single-pass bf16 matmuls (no hi/lo compensation): Bu is 8 accumulating
    PE matmuls per 512-wide l-tile; the readout y = h_re^T @ C^T runs
    straight off the [64-state, l] post-rotation output as K=64 matmuls.
  * The complex scan is rotated into a per-lane REAL damped scan:
    g_t = r*g_{t-1} + w_t with w_t = e^{-i theta t} Bu_t (elementwise
    rotation against host cos/sin tables in bf16), h_re_t = Re(e^{i theta t}
    g_t), mapped to one DVE tensor_tensor_scan per l-tile, chained via its
    initial value.
  * Rotation mults run on POOL reading Bu directly from PSUM; the rotation
    add/sub and the scan run on DVE. Residual x = y + u with the LN sum
    accumulator is split across DVE (first d-half) and POOL (second half);
    sum(x^2) and the final normalize run on ACT.
  * The output is written bf16 and upcast to fp32 on the host.
  * The l-tile loop is software-pipelined (Bu of tile i is emitted two
    tiles ahead of its scan/readout) to keep the PE queue free of
    head-of-line waits; DMA is batched per l-tile via 3D access patterns.
"""

import numpy as np

import concourse.mybir as mybir
import concourse.tile as tile
from concourse import bacc, bass_utils
from concourse.bass import MemorySpace
from concourse.mybir import ActivationFunctionType as act
from concourse.mybir import AluOpType as alu

F32 = mybir.dt.float32
BF16 = mybir.dt.bfloat16
P = 128          # partitions
L = 4096         # sequence length per core
DM = 1024        # d_model
NS = 64          # d_state
LT = 512         # l-tile (scan slice, matmul moving width)
NSUB = LT // P   # 4 l-subtiles of 128 rows per l-tile
NT = L // LT     # 8 l-tiles
KC = DM // P     # 8 contraction chunks of 128
NCORES = 8
LN_EPS = 1e-5
DH = 512         # d-model half (psum bank width)
DEPTH = 2        # software pipeline: Bu runs DEPTH l-tiles ahead


def _build_program(use_gb: bool):
    """Builds the single-core Bass/Tile program (SPMD across 8 cores)."""
    nc = bacc.Bacc("TRN2", num_devices=NCORES, debug=False)

    # host-pre-tiled layouts: every per-l-tile DMA slice is fully contiguous
    # per partition (8 KiB runs) for peak HBM efficiency
    un_d = nc.dram_tensor("un", [P, NT, NSUB, DM], BF16, kind="ExternalInput").ap()
    uth_d = nc.dram_tensor("uth", [P, NT, KC, LT], BF16, kind="ExternalInput").ap()
    bb_d = nc.dram_tensor("bb", [P, DM], BF16, kind="ExternalInput").ap()
    ct_d = nc.dram_tensor("ct", [NS, DM], BF16, kind="ExternalInput").ap()
    trig_d = nc.dram_tensor("trig", [P, L], BF16, kind="ExternalInput").ap()
    trigb_d = nc.dram_tensor("trigb", [P, L], BF16, kind="ExternalInput").ap()
    rt_d = nc.dram_tensor("rt", [P, LT], F32, kind="ExternalInput").ap()
    if use_gb:
        gam_d = nc.dram_tensor("gam", [P, DM], F32, kind="ExternalInput").ap()
        bet_d = nc.dram_tensor("bet", [P, DM], F32, kind="ExternalInput").ap()
    out_d = nc.dram_tensor("out", [P, NT, NSUB, DM], BF16, kind="ExternalOutput").ap()

    with tile.TileContext(nc) as tc:
        with (
            tc.tile_pool(name="singles", bufs=1) as singles,
            tc.tile_pool(name="un", bufs=2) as un_pool,
            tc.tile_pool(name="ut", bufs=2) as ut_pool,
            tc.tile_pool(name="w", bufs=2) as w_pool,
            tc.tile_pool(name="g", bufs=2) as g_pool,
            tc.tile_pool(name="h", bufs=3) as h_pool,
            tc.tile_pool(name="x", bufs=6) as x_pool,
            tc.tile_pool(name="tmp", bufs=3) as tmp_pool,
            tc.tile_pool(name="o", bufs=2) as o_pool,
            tc.tile_pool(name="st", bufs=3) as st_pool,
            tc.tile_pool(name="pb", bufs=3, space=MemorySpace.PSUM) as psum_b,
            tc.tile_pool(name="py", bufs=2, space=MemorySpace.PSUM) as psum_y,
        ):
            bb_s = singles.tile([P, DM], BF16)
            nc.sync.dma_start(bb_s[:], bb_d)
            ct_s = singles.tile([NS, DM], BF16)
            nc.sync.dma_start(ct_s[:], ct_d)
            trig = singles.tile([P, L], BF16)
            nc.sync.dma_start(trig[:], trig_d)
            trigb = singles.tile([P, L], BF16)
            nc.sync.dma_start(trigb[:], trigb_d)
            rt_s = singles.tile([P, LT], F32)
            nc.sync.dma_start(rt_s[:], rt_d)
            eps_s = singles.tile([P, 1], F32)
            nc.gpsimd.memset(eps_s[:], LN_EPS)
            if use_gb:
                gam_s = singles.tile([P, DM], F32)
                nc.sync.dma_start(gam_s[:], gam_d)
                bet_s = singles.tile([P, DM], F32)
                nc.sync.dma_start(bet_s[:], bet_d)

            g_prev = None
            stash = {}
            hb_stash = {}
            for it in range(NT + DEPTH + 1):
                # ---- stage A: load uT + Bu matmul for tile `it` ----
                if it < NT:
                    th_t = ut_pool.tile([P, KC, LT], BF16, tag="uth")
                    nc.sync.dma_start(th_t[:], uth_d[:, it])

                    bu = psum_b.tile([P, LT], F32, tag="bu")
                    for k in range(KC):
                        nc.tensor.matmul(
                            bu[:],
                            bb_s[:, k * P : (k + 1) * P],
                            th_t[:, k, :],
                            start=(k == 0),
                            stop=(k == KC - 1),
                        )
                    stash[it] = bu

                # ---- stage B1: rotation + scan for tile `it-DEPTH` ----
                # Emitted ahead of stage B2 so every engine's in-order queue
                # runs the next tile's scan pipeline before this tile's LN
                # tail; otherwise the scan chain serializes on the tail.
                j1 = it - DEPTH
                if 0 <= j1 < NT:
                    bu = stash.pop(j1)
                    l0 = j1 * LT
                    # trig: cos on parts 0-63, sin on 64-127; trigb swapped.
                    cs_lo = trig[0:NS, l0 : l0 + LT]
                    sn_hi = trig[NS:P, l0 : l0 + LT]
                    sn_lo = trigb[0:NS, l0 : l0 + LT]
                    cs_hi = trigb[NS:P, l0 : l0 + LT]

                    # pre-rotation: w = e^{-i theta t} * Bu.  GPSIMD cannot
                    # read PSUM, so ACT stages Bu into SBUF first.
                    bs = w_pool.tile([P, LT], F32, tag="bs")
                    nc.scalar.copy(bs[:], bu[:])
                    w = w_pool.tile([P, LT], F32, tag="w")
                    t1 = tmp_pool.tile([NS, LT], F32, tag="t1")
                    t2 = tmp_pool.tile([NS, LT], F32, tag="t2")
                    nc.gpsimd.tensor_tensor(t1[:], bs[0:NS, :], cs_lo, alu.mult)
                    nc.gpsimd.tensor_tensor(t2[:], bs[NS:P, :], sn_hi, alu.mult)
                    nc.gpsimd.tensor_tensor(w[0:NS, :], t1[:], t2[:], alu.add)
                    t3 = tmp_pool.tile([NS, LT], F32, tag="t1")
                    t4 = tmp_pool.tile([NS, LT], F32, tag="t2")
                    nc.gpsimd.tensor_tensor(t3[:], bs[NS:P, :], cs_hi, alu.mult)
                    nc.vector.tensor_tensor(t4[:], bu[0:NS, :], sn_lo, alu.mult)
                    nc.gpsimd.tensor_tensor(w[NS:P, :], t3[:], t4[:], alu.subtract)

                    # damped real scan (DVE), chained across l-tiles
                    g = g_pool.tile([P, LT], F32, tag="g")
                    init = 0.0 if g_prev is None else g_prev[:, LT - 1 : LT]
                    nc.vector.tensor_tensor_scan(
                        g[:], rt_s[:], w[:], init, alu.mult, alu.add
                    )
                    g_prev = g

                    # post-rotation h_re = cos*g_re - sin*g_im, bf16 for PE
                    t5 = tmp_pool.tile([NS, LT], F32, tag="t1")
                    t6 = tmp_pool.tile([NS, LT], F32, tag="t2")
                    nc.gpsimd.tensor_tensor(t5[:], g[0:NS, :], cs_lo, alu.mult)
                    nc.gpsimd.tensor_tensor(t6[:], g[NS:P, :], sn_hi, alu.mult)
                    hb = h_pool.tile([NS, LT], BF16, tag="hb")
                    nc.vector.tensor_tensor(hb[:], t5[:], t6[:], alu.subtract)
                    hb_stash[j1] = hb

                # ---- stage B2: readout + residual + LN for `it-DEPTH-1` ----
                jt = it - DEPTH - 1
                if jt < 0:
                    continue
                hb = hb_stash.pop(jt)
                un_t = un_pool.tile([P, NSUB, DM], BF16, tag="un")
                nc.sync.dma_start(un_t[:], un_d[:, jt])

                # readout + residual + LN stats per 128-row l-subtile.
                # y2 spans two PSUM banks (each matmul writes within one);
                # one fused [128, DM] residual STT per subtile on DVE.
                sx = st_pool.tile([P, NSUB], F32, tag="sx")
                sq = st_pool.tile([P, NSUB], F32, tag="sq")
                x_list = []
                for ls in range(NSUB):
                    lhsT = hb[:, ls * P : (ls + 1) * P]
                    x = x_pool.tile([P, DM], F32, tag="x")
                    y2 = psum_y.tile([P, DM], F32, tag="y")
                    for dh in range(2):
                        sl = slice(dh * DH, (dh + 1) * DH)
                        nc.tensor.matmul(
                            y2[:, sl], lhsT, ct_s[:, sl], start=True, stop=True
                        )
                    nc.vector.scalar_tensor_tensor(
                        x[:],
                        y2[:],
                        1.0,
                        un_t[:, ls, :],
                        alu.mult,
                        alu.add,
                        accum_out=sx[:, ls : ls + 1],
                    )
                    sqs = tmp_pool.tile([P, DM], F32, tag="sqs")
                    nc.scalar.activation(
                        sqs[:], x[:], act.Square, accum_out=sq[:, ls : ls + 1]
                    )
                    x_list.append(x)

                # LN stats for the 4 l-subtiles
                mu = st_pool.tile([P, NSUB], F32, tag="mu")
                nc.scalar.mul(mu[:], sx[:], 1.0 / DM)
                ex2 = st_pool.tile([P, NSUB], F32, tag="ex2")
                nc.scalar.mul(ex2[:], sq[:], 1.0 / DM)
                var = st_pool.tile([P, NSUB], F32, tag="var")
                nc.vector.tensor_tensor(var[:], mu[:], mu[:], alu.mult)
                nc.vector.tensor_tensor(var[:], ex2[:], var[:], alu.subtract)
                sd = st_pool.tile([P, NSUB], F32, tag="sd")
                nc.scalar.activation(sd[:], var[:], act.Sqrt, bias=eps_s[:, 0:1])
                rstd = st_pool.tile([P, NSUB], F32, tag="rstd")
                nc.vector.reciprocal(rstd[:], sd[:])
                nmr = st_pool.tile([P, NSUB], F32, tag="nmr")
                nc.vector.scalar_tensor_tensor(
                    nmr[:], mu[:], -1.0, rstd[:], alu.mult, alu.mult
                )

                # normalize on ACT: o = x*rstd + (-mu*rstd); batched bf16 store
                o_t = o_pool.tile([P, NSUB, DM], BF16, tag="o")
                for ls in range(NSUB):
                    nc.scalar.activation(
                        o_t[:, ls, :],
                        x_list[ls][:],
                        act.Identity,
                        bias=nmr[:, ls : ls + 1],
                        scale=rstd[:, ls : ls + 1],
                    )
                    if use_gb:
                        nc.vector.tensor_tensor(
                            o_t[:, ls, :], o_t[:, ls, :], gam_s[:], alu.mult
                        )
                        nc.vector.tensor_tensor(
                            o_t[:, ls, :], o_t[:, ls, :], bet_s[:], alu.add
                        )
                nc.sync.dma_start(out_d[:, jt], o_t[:])
    nc.compile()
    return nc


try:
    import ml_dtypes

    ml_bf16 = ml_dtypes.bfloat16
except ImportError:  # pragma: no cover
    ml_bf16 = None


def _host_params(log_neg_real, imag, B_mat, C_mat):
    lnr = np.asarray(log_neg_real, np.float64)
    im = np.asarray(imag, np.float64)
    a = -np.exp(lnr) + 1j * im
    a_bar = np.exp(a)
    r = np.abs(a_bar)
    b_bar = ((a_bar - 1.0) / a)[:, None] * np.asarray(B_mat, np.float64)
    b_re = np.real(b_bar).astype(np.float32)
    b_im = np.imag(b_bar).astype(np.float32)
    # packed stationary operand for the Bu matmul: [K=d, M=128(re|im)] laid out
    # in SBUF as [128 partitions, KC*128] with chunk k at columns k*128:(k+1)*128
    bbT = np.concatenate([b_re, b_im], axis=0).T  # (DM, 128)
    bb = np.ascontiguousarray(
        bbT.reshape(KC, P, P).transpose(1, 0, 2).reshape(P, DM)
    ).astype(ml_bf16)
    ct = np.ascontiguousarray(np.asarray(C_mat, np.float32).T).astype(ml_bf16)
    t = np.arange(L, dtype=np.float64)
    ang = (im[:, None] * t[None, :]) % (2 * np.pi)
    cosT = np.cos(ang).astype(np.float32)
    sinT = np.sin(ang).astype(np.float32)
    trig = np.ascontiguousarray(np.concatenate([cosT, sinT], axis=0)).astype(
        ml_bf16
    )  # (128, L)
    trigb = np.ascontiguousarray(np.concatenate([sinT, cosT], axis=0)).astype(
        ml_bf16
    )
    rfull = np.concatenate([r, r]).astype(np.float32)
    rt = np.ascontiguousarray(np.broadcast_to(rfull[:, None], (P, LT)))
    return bb, ct, trig, trigb, rt


_PROGRAM_CACHE = {}


def kernel(u, log_neg_real, imag, B_mat, C_mat, D, gamma, beta):
    _cache = _PROGRAM_CACHE
    u = np.ascontiguousarray(np.asarray(u, np.float32))
    Dv = np.asarray(D, np.float32)
    gam = np.asarray(gamma, np.float32)
    bet = np.asarray(beta, np.float32)
    use_ures = bool(np.any(Dv != 0.0))
    use_gb = bool(np.any(gam != 1.0) or np.any(bet != 0.0))

    bb, ct, trig, trigb, rt = _host_params(log_neg_real, imag, B_mat, C_mat)

    if use_gb not in _cache:
        _cache[use_gb] = _build_program(use_gb)
    nc = _cache[use_gb]

    shared = {"bb": bb, "ct": ct, "trig": trig, "trigb": trigb, "rt": rt}
    if use_gb:
        shared["gam"] = np.ascontiguousarray(
            np.broadcast_to(gam[None, :], (P, DM)).astype(np.float32)
        )
        shared["bet"] = np.ascontiguousarray(
            np.broadcast_to(bet[None, :], (P, DM)).astype(np.float32)
        )
    in_maps = []
    for b in range(NCORES):
        m = dict(shared)
        ub = u[b]
        ures = ub * (1.0 + Dv)[None, :] if use_ures else ub
        # pre-tiled [P, NT, NSUB, DM]: l = it*LT + s*128 + p
        m["un"] = np.ascontiguousarray(
            ures.astype(ml_bf16).reshape(NT, NSUB, P, DM).transpose(2, 0, 1, 3)
        )
        # pre-tiled [P, NT, KC, LT]: d = c*128 + p, l = it*LT + j
        m["uth"] = np.ascontiguousarray(
            ub.T.astype(ml_bf16).reshape(KC, P, NT, LT).transpose(1, 2, 0, 3)
        )
        in_maps.append(m)

    res = bass_utils.run_bass_kernel_spmd(nc, in_maps, core_ids=list(range(NCORES)))
    # un-permute [P, NT, NSUB, DM] -> [L, DM] and upcast
    return np.stack(
        [
            r["out"].transpose(1, 2, 0, 3).reshape(L, DM).astype(np.float32)
            for r in res.results
        ],
        axis=0,
    )


# revision 29
# speedup vs baseline: 1.2615x; 1.2615x over previous
"""Trainium2 Bass kernel for the DiagonalSSMBlock problem.

Math (per batch, sharded one batch per core over 8 cores):
    a = -exp(log_neg_real) + i*imag ; a_bar = exp(a) = r * e^{i theta}
    b_bar = ((a_bar-1)/a)[:,None] * B
    Bu_t = b_bar @ u_t                         (complex, state dim 64)
    h_t = a_bar * h_{t-1} + Bu_t               (diagonal complex scan over L)
    y_t = Re(C @ h_t) + D*u_t ; out = LN(u + y) * gamma + beta

The kernel is DMA-bound, so all HBM traffic runs in bf16 (the correctness
gate is rel-err < 2e-2; the all-bf16 pipeline sits at ~2.4e-3):
  * u ships twice from host in bf16: transposed [d, l] for the Bu matmul
    (contraction over d_model needs d on partitions) and natural [l, d] for
    the residual (pre-scaled by (1+D) on host); both pre-tiled so every
    per-tile DMA is fully contiguous per partition.  The output is written bf16 and
    un-permuted/upcast on the host.
  * Single-pass bf16 matmuls (no hi/lo compensation).
  * The complex scan is rotated into a per-lane REAL damped scan
    g_t = r*g_{t-1} + w_t with w_t = e^{-i theta t} Bu_t, one DVE
    tensor_tensor_scan per 512-wide l-tile, chained via its initial value.
    The rotations use stacked full-height [128, LT] elementwise ops:
      pre:  m1 = [cos;sin].*Bu, m2 = [sin;cos].*Bu (DVE, reading PSUM),
            w_lo = m1_lo + m1_hi, w_hi = m2_hi - m2_lo (POOL)
      post: G = [cos;sin].*g (POOL, bf16) and the readout contracts G
            against ct2 = [C^T; -C^T] with K=128, absorbing the
            h_re = cos*g_re - sin*g_im combination into the PE.
  * Residual x = y2 + u runs as one fused [128, d_model] DVE STT per
    l-subtile with the LN row-sum accumulated on the fly (y2 spans two
    PSUM banks); sum(x^2) via ACT Square accumulate; LN scalar stats on
    POOL with a single ACT Rsqrt; normalize on ACT with per-partition
    bias/scale.
  * Three-stage software pipeline per l-tile (A: DMA+Bu matmul two tiles
    ahead; B1: rotation+scan one tile ahead; B2: readout+residual+LN+store)
    so each engine's in-order queue runs next-tile scan work before the
    current tile's LN tail.
"""

import numpy as np

import concourse.mybir as mybir
import concourse.tile as tile
from concourse import bacc, bass_utils
from concourse.bass import MemorySpace
from concourse.mybir import ActivationFunctionType as act
from concourse.mybir import AluOpType as alu

F32 = mybir.dt.float32
BF16 = mybir.dt.bfloat16
P = 128          # partitions
L = 4096         # sequence length per core
DM = 1024        # d_model
NS = 64          # d_state
LT = 512         # l-tile (scan slice, matmul moving width)
NSUB = LT // P   # 4 l-subtiles of 128 rows per l-tile
NT = L // LT     # 8 l-tiles
KC = DM // P     # 8 contraction chunks of 128
NCORES = 8
LN_EPS = 1e-5
DH = 512         # d-model half (psum bank width)
DEPTH = 1        # stage A runs DEPTH l-tiles ahead of B1


def _build_program(use_gb: bool):
    """Builds the single-core Bass/Tile program (SPMD across 8 cores)."""
    nc = bacc.Bacc("TRN2", num_devices=NCORES, debug=False)

    # host-pre-tiled layouts: every per-l-tile DMA slice is fully contiguous
    # per partition for peak HBM efficiency
    un_d = nc.dram_tensor("un", [P, NT, NSUB, DM], BF16, kind="ExternalInput").ap()
    uth_d = nc.dram_tensor("uth", [P, NT, KC, LT], BF16, kind="ExternalInput").ap()
    bb_d = nc.dram_tensor("bb", [P, DM], BF16, kind="ExternalInput").ap()
    ct2_d = nc.dram_tensor("ct2", [P, DM], BF16, kind="ExternalInput").ap()
    smix_d = nc.dram_tensor("smix", [P, P], BF16, kind="ExternalInput").ap()
    trig_d = nc.dram_tensor("trig", [P, L], BF16, kind="ExternalInput").ap()
    trigb_d = nc.dram_tensor("trigb", [P, L], BF16, kind="ExternalInput").ap()
    rt_d = nc.dram_tensor("rt", [P, LT], F32, kind="ExternalInput").ap()
    if use_gb:
        gam_d = nc.dram_tensor("gam", [P, DM], F32, kind="ExternalInput").ap()
        bet_d = nc.dram_tensor("bet", [P, DM], F32, kind="ExternalInput").ap()
    out_d = nc.dram_tensor("out", [P, NT, NSUB, DM], BF16, kind="ExternalOutput").ap()

    with tile.TileContext(nc) as tc:
        with (
            tc.tile_pool(name="singles", bufs=1) as singles,
            tc.tile_pool(name="un", bufs=2) as un_pool,
            tc.tile_pool(name="ut", bufs=3) as ut_pool,
            tc.tile_pool(name="tg", bufs=4) as tg_pool,
            tc.tile_pool(name="m", bufs=2) as m_pool,
            tc.tile_pool(name="g", bufs=3) as g_pool,
            tc.tile_pool(name="gg", bufs=2) as gg_pool,
            tc.tile_pool(name="x", bufs=6) as x_pool,
            tc.tile_pool(name="sqs", bufs=2) as sqs_pool,
            tc.tile_pool(name="o", bufs=2) as o_pool,
            tc.tile_pool(name="st", bufs=3) as st_pool,
            tc.tile_pool(name="pb", bufs=2, space=MemorySpace.PSUM) as psum_b,
            tc.tile_pool(name="pw", bufs=2, space=MemorySpace.PSUM) as psum_w,
            tc.tile_pool(name="py", bufs=2, space=MemorySpace.PSUM) as psum_y,
        ):
            bb_s = singles.tile([P, DM], BF16)
            nc.sync.dma_start(bb_s[:], bb_d)
            ct2_s = singles.tile([P, DM], BF16)
            nc.sync.dma_start(ct2_s[:], ct2_d)
            smix_s = singles.tile([P, P], BF16)
            nc.sync.dma_start(smix_s[:], smix_d)
            rt_s = singles.tile([P, LT], F32)
            nc.sync.dma_start(rt_s[:], rt_d)
            eps_s = singles.tile([P, 1], F32)
            nc.gpsimd.memset(eps_s[:], LN_EPS)
            if use_gb:
                gam_s = singles.tile([P, DM], F32)
                nc.sync.dma_start(gam_s[:], gam_d)
                bet_s = singles.tile([P, DM], F32)
                nc.sync.dma_start(bet_s[:], bet_d)

            g_prev = None
            stash = {}
            for it in range(NT + DEPTH + 1):
                # ---- stage A: load uT + trig slices + Bu matmul, tile `it`
                if it < NT:
                    l0 = it * LT
                    th_t = ut_pool.tile([P, KC, LT], BF16, tag="uth")
                    nc.sync.dma_start(th_t[:], uth_d[:, it])
                    tg_t = tg_pool.tile([P, LT], BF16, tag="tg")
                    nc.sync.dma_start(tg_t[:], trig_d[:, l0 : l0 + LT])
                    tgb_t = tg_pool.tile([P, LT], BF16, tag="tgb")
                    nc.sync.dma_start(tgb_t[:], trigb_d[:, l0 : l0 + LT])

                    bu = psum_b.tile([P, LT], F32, tag="bu")
                    for k in range(KC):
                        nc.tensor.matmul(
                            bu[:],
                            bb_s[:, k * P : (k + 1) * P],
                            th_t[:, k, :],
                            start=(k == 0),
                            stop=(k == KC - 1),
                        )
                    stash[it] = (bu, tg_t, tgb_t)

                # ---- stage B1: pre-rotation + scan for tile `it-DEPTH` ----
                # Emitted ahead of stage B2 so every engine's in-order queue
                # runs the next tile's scan pipeline before this tile's LN
                # tail; otherwise the scan chain serializes on the tail.
                j1 = it - DEPTH
                if 0 <= j1 < NT:
                    bu, tg_t, tgb_t = stash[j1]
                    # m1 = [cos;sin]*Bu, m2 = [sin;cos]*Bu (DVE reads PSUM,
                    # bf16 out so the half-mix runs as cheap bf16 matmuls)
                    m1 = m_pool.tile([P, LT], BF16, tag="m1")
                    nc.vector.tensor_tensor(m1[:], tg_t[:], bu[:], alu.mult)
                    m2 = m_pool.tile([P, LT], BF16, tag="m2")
                    nc.vector.tensor_tensor(m2[:], tgb_t[:], bu[:], alu.mult)
                    # half-mix on PE: w_lo = m1_lo + m1_hi, w_hi = m2_hi-m2_lo
                    # via stationary +-identity stacks (cross-partition adds
                    # are not expressible as single DVE/POOL ops)
                    w = psum_w.tile([P, LT], F32, tag="w")
                    nc.tensor.matmul(
                        w[0:NS, :], smix_s[:, 0:NS], m1[:], start=True, stop=True
                    )
                    nc.tensor.matmul(
                        w[NS:P, :], smix_s[:, NS:P], m2[:], start=True, stop=True
                    )
                    # damped real scan (DVE), chained across l-tiles
                    g = g_pool.tile([P, LT], F32, tag="g")
                    init = 0.0 if g_prev is None else g_prev[:, LT - 1 : LT]
                    nc.vector.tensor_tensor_scan(
                        g[:], rt_s[:], w[:], init, alu.mult, alu.add
                    )
                    g_prev = g
                    stash[j1] = (g, tg_t)

                # ---- stage B2: readout + residual + LN for `it-DEPTH-1` ---
                jt = it - DEPTH - 1
                if jt < 0:
                    continue
                g, tg_t = stash.pop(jt)
                un_t = un_pool.tile([P, NSUB, DM], BF16, tag="un")
                nc.sync.dma_start(un_t[:], un_d[:, jt])

                # post-rotation G = [cos;sin]*g in bf16; the PE absorbs
                # h_re = cos*g_re - sin*g_im via ct2 = [C^T; -C^T]
                gb = gg_pool.tile([P, LT], BF16, tag="gb")
                nc.gpsimd.tensor_tensor(gb[:], tg_t[:], g[:], alu.mult)

                # readout + residual + LN stats per 128-row l-subtile.
                # y2 spans two PSUM banks (each matmul writes within one);
                # one fused [128, DM] residual STT per subtile on DVE.
                sx = st_pool.tile([P, NSUB], F32, tag="sx")
                sq = st_pool.tile([P, NSUB], F32, tag="sq")
                x_list = []
                for ls in range(NSUB):
                    lhsT = gb[:, ls * P : (ls + 1) * P]
                    x = x_pool.tile([P, DM], F32, tag="x")
                    y2 = psum_y.tile([P, DM], F32, tag="y")
                    for dh in range(2):
                        sl = slice(dh * DH, (dh + 1) * DH)
                        nc.tensor.matmul(
                            y2[:, sl], lhsT, ct2_s[:, sl], start=True, stop=True
                        )
                    nc.vector.scalar_tensor_tensor(
                        x[:],
                        y2[:],
                        1.0,
                        un_t[:, ls, :],
                        alu.mult,
                        alu.add,
                        accum_out=sx[:, ls : ls + 1],
                    )
                    sqs = sqs_pool.tile([P, DM], F32, tag="sqs")
                    nc.scalar.activation(
                        sqs[:], x[:], act.Square, accum_out=sq[:, ls : ls + 1]
                    )
                    x_list.append(x)

                # LN stats: mu = sx/DM ; var = sq/DM - mu^2 ; rstd ; -mu*rstd
                mu = st_pool.tile([P, NSUB], F32, tag="mu")
                nc.gpsimd.tensor_scalar_mul(mu[:], sx[:], 1.0 / DM)
                ex2 = st_pool.tile([P, NSUB], F32, tag="ex2")
                nc.gpsimd.tensor_scalar_mul(ex2[:], sq[:], 1.0 / DM)
                var = st_pool.tile([P, NSUB], F32, tag="var")
                nc.gpsimd.tensor_tensor(var[:], mu[:], mu[:], alu.mult)
                nc.gpsimd.tensor_tensor(var[:], ex2[:], var[:], alu.subtract)
                sd = st_pool.tile([P, NSUB], F32, tag="sd")
                nc.scalar.activation(sd[:], var[:], act.Sqrt, bias=eps_s[:, 0:1])
                rstd = st_pool.tile([P, NSUB], F32, tag="rstd")
                nc.vector.reciprocal(rstd[:], sd[:])
                nmr = st_pool.tile([P, NSUB], F32, tag="nmr")
                nc.vector.scalar_tensor_tensor(
                    nmr[:], mu[:], -1.0, rstd[:], alu.mult, alu.mult
                )

                # normalize: o = x*rstd + (-mu*rstd); 3 subtiles on ACT, one
                # on POOL (two-scalar tensor_scalar) to balance engine load
                o_t = o_pool.tile([P, NSUB, DM], BF16, tag="o")
                for ls in range(NSUB):
                    if ls == NSUB - 1:
                        nc.vector.tensor_scalar(
                            o_t[:, ls, :],
                            x_list[ls][:],
                            rstd[:, ls : ls + 1],
                            nmr[:, ls : ls + 1],
                            alu.mult,
                            alu.add,
                        )
                    else:
                        nc.scalar.activation(
                            o_t[:, ls, :],
                            x_list[ls][:],
                            act.Identity,
                            bias=nmr[:, ls : ls + 1],
                            scale=rstd[:, ls : ls + 1],
                        )
                    if use_gb:
                        nc.vector.tensor_tensor(
                            o_t[:, ls, :], o_t[:, ls, :], gam_s[:], alu.mult
                        )
                        nc.vector.tensor_tensor(
                            o_t[:, ls, :], o_t[:, ls, :], bet_s[:], alu.add
                        )
                nc.sync.dma_start(out_d[:, jt], o_t[:])
    nc.compile()
    return nc


try:
    import ml_dtypes

    ml_bf16 = ml_dtypes.bfloat16
except ImportError:  # pragma: no cover
    ml_bf16 = None


def _host_params(log_neg_real, imag, B_mat, C_mat):
    lnr = np.asarray(log_neg_real, np.float64)
    im = np.asarray(imag, np.float64)
    a = -np.exp(lnr) + 1j * im
    a_bar = np.exp(a)
    r = np.abs(a_bar)
    b_bar = ((a_bar - 1.0) / a)[:, None] * np.asarray(B_mat, np.float64)
    b_re = np.real(b_bar).astype(np.float32)
    b_im = np.imag(b_bar).astype(np.float32)
    # packed stationary operand for the Bu matmul: [K=d, M=128(re|im)] laid out
    # in SBUF as [128 partitions, KC*128] with chunk k at columns k*128:(k+1)*128
    bbT = np.concatenate([b_re, b_im], axis=0).T  # (DM, 128)
    bb = np.ascontiguousarray(
        bbT.reshape(KC, P, P).transpose(1, 0, 2).reshape(P, DM)
    ).astype(ml_bf16)
    ct = np.asarray(C_mat, np.float32).T  # (NS, DM)
    ct2 = np.ascontiguousarray(np.concatenate([ct, -ct], axis=0)).astype(ml_bf16)
    # half-mix stationaries: cols 0:64 -> w_lo = m1_lo + m1_hi,
    # cols 64:128 -> w_hi = m2_hi - m2_lo (out partitions 64..127)
    eye = np.eye(NS, dtype=np.float32)
    smix = np.zeros((P, P), np.float32)
    smix[0:NS, 0:NS] = eye
    smix[NS:P, 0:NS] = eye
    smix[0:NS, NS:P] = -eye
    smix[NS:P, NS:P] = eye
    smix = np.ascontiguousarray(smix).astype(ml_bf16)
    t = np.arange(L, dtype=np.float64)
    ang = (im[:, None] * t[None, :]) % (2 * np.pi)
    cosT = np.cos(ang).astype(np.float32)
    sinT = np.sin(ang).astype(np.float32)
    trig = np.ascontiguousarray(np.concatenate([cosT, sinT], axis=0)).astype(
        ml_bf16
    )  # (128, L)
    trigb = np.ascontiguousarray(np.concatenate([sinT, cosT], axis=0)).astype(
        ml_bf16
    )
    rfull = np.concatenate([r, r]).astype(np.float32)
    rt = np.ascontiguousarray(np.broadcast_to(rfull[:, None], (P, LT)))
    return bb, ct2, smix, trig, trigb, rt


_PROGRAM_CACHE = {}


def kernel(u, log_neg_real, imag, B_mat, C_mat, D, gamma, beta):
    _cache = _PROGRAM_CACHE
    u = np.ascontiguousarray(np.asarray(u, np.float32))
    Dv = np.asarray(D, np.float32)
    gam = np.asarray(gamma, np.float32)
    bet = np.asarray(beta, np.float32)
    use_ures = bool(np.any(Dv != 0.0))
    use_gb = bool(np.any(gam != 1.0) or np.any(bet != 0.0))

    bb, ct2, smix, trig, trigb, rt = _host_params(log_neg_real, imag, B_mat, C_mat)

    if use_gb not in _cache:
        _cache[use_gb] = _build_program(use_gb)
    nc = _cache[use_gb]

    shared = {
        "bb": bb, "ct2": ct2, "smix": smix,
        "trig": trig, "trigb": trigb, "rt": rt,
    }
    if use_gb:
        shared["gam"] = np.ascontiguousarray(
            np.broadcast_to(gam[None, :], (P, DM)).astype(np.float32)
        )
        shared["bet"] = np.ascontiguousarray(
            np.broadcast_to(bet[None, :], (P, DM)).astype(np.float32)
        )
    in_maps = []
    for b in range(NCORES):
        m = dict(shared)
        ub = u[b]
        ures = ub * (1.0 + Dv)[None, :] if use_ures else ub
        # pre-tiled [P, NT, NSUB, DM]: l = it*LT + s*128 + p
        m["un"] = np.ascontiguousarray(
            ures.astype(ml_bf16).reshape(NT, NSUB, P, DM).transpose(2, 0, 1, 3)
        )
        # pre-tiled [P, NT, KC, LT]: d = c*128 + p, l = it*LT + j
        m["uth"] = np.ascontiguousarray(
            ub.T.astype(ml_bf16).reshape(KC, P, NT, LT).transpose(1, 2, 0, 3)
        )
        in_maps.append(m)

    res = bass_utils.run_bass_kernel_spmd(nc, in_maps, core_ids=list(range(NCORES)))
    # un-permute [P, NT, NSUB, DM] -> [L, DM] and upcast
    return np.stack(
        [
            r["out"].transpose(1, 2, 0, 3).reshape(L, DM).astype(np.float32)
            for r in res.results
        ],
        axis=0,
    )
